# revision 4
# baseline (speedup 1.0000x reference)
"""Trainium2 Bass kernel for the DenseSNN problem (4-layer LIF spiking MLP).

Strategy
--------
Data-parallel over batch: B=128 is split into 8 shards of 16, one per
NeuronCore, with weights replicated (no collectives at all).

Per core the time recurrence is restructured layer-at-a-time: layer l's
input spikes for ALL timesteps are known once layer l-1's LIF scan
finishes, so each layer becomes ONE batched matmul over all (t, b) pairs
(M = T*Bs = 1024 rows -> full PE utilization) followed by a sequential
64-step elementwise LIF scan on the Vector engine:

    mem   = beta*mem + cur[t] - th*spk[t-1]     (2x scalar_tensor_tensor)
    spk[t] = (mem > th)                          (tensor_scalar is_gt)

All matmul operands are bf16 (1 cycle/row on the PE vs 4 for fp32);
accumulation stays fp32 in PSUM. Spikes are exactly representable in
bf16 (0.0/1.0). The per-neuron bias is fused into the PSUM->SBUF
evacuation on the Scalar engine (activation Identity with a bias AP).

Layers are processed in two 512-column chunks (tc) so the next chunk's /
next layer's matmuls overlap the previous chunk's LIF scan.

Layout (per core)
-----------------
Columns are (t, b) pairs, t-major: col = t*16 + b, 1024 columns total.
Activations/spikes live as [128 partitions, kt, col]: row kt*128+p of the
conceptual [D, cols] matrix sits at partition p, block kt.
Weights are pre-transposed + blocked host-side to [p, mt, kt, f] so each
matmul lhsT tile is w[:, mt, kt, :] = [K=128, M=128] and a whole mt-chunk
is one contiguous DMA.
"""

import os
import sys

import numpy as np
import ml_dtypes

if "/opt/trn_rl_repo" not in sys.path:
    sys.path.insert(0, "/opt/trn_rl_repo")

T, B, D_IN, D_H, D_OUT = 64, 128, 1024, 2048, 1000
NCORES = 8
BS = B // NCORES           # 16 batch rows per core
COLS = T * BS              # 1024 (t, b) columns
NTC = 2                    # column chunks per layer
CHW = COLS // NTC          # 512 columns per chunk (one PSUM bank)
TPC = T // NTC             # timesteps per chunk

BF16 = ml_dtypes.bfloat16

_COMPILED = {}


# --------------------------------------------------------------------------
# Program construction
# --------------------------------------------------------------------------

def _build(params, debug=False):
    from concourse import bacc, tile, mybir

    beta1, th1, beta2, th2, beta3, th3, beta_o, th_o = params
    f32 = mybir.dt.float32
    bf = mybir.dt.bfloat16
    Al = mybir.AluOpType
    AF = mybir.ActivationFunctionType

    nc = bacc.Bacc(
        "TRN2", target_bir_lowering=False, debug=False, num_devices=NCORES
    )

    xT_d = nc.dram_tensor("xT", [128, 8, COLS], bf, kind="ExternalInput")
    w1_d = nc.dram_tensor("w1T", [128, 16, 8, 128], bf, kind="ExternalInput")
    w2_d = nc.dram_tensor("w2T", [128, 16, 16, 128], bf, kind="ExternalInput")
    w3_d = nc.dram_tensor("w3T", [128, 16, 16, 128], bf, kind="ExternalInput")
    wo_d = nc.dram_tensor("woT", [128, 8, 16, 128], bf, kind="ExternalInput")
    b1_d = nc.dram_tensor("b1v", [128, 16], f32, kind="ExternalInput")
    b2_d = nc.dram_tensor("b2v", [128, 16], f32, kind="ExternalInput")
    b3_d = nc.dram_tensor("b3v", [128, 16], f32, kind="ExternalInput")
    bo_d = nc.dram_tensor("bov", [128, 8], f32, kind="ExternalInput")
    out_d = nc.dram_tensor("acc_out", [128, 8, BS], f32, kind="ExternalOutput")
    if debug:
        dbg_d = nc.dram_tensor("dbg_s", [128, 3, 16], f32, kind="ExternalOutput")

    with tile.TileContext(nc) as tc:
        with (
            tc.tile_pool(name="const", bufs=1) as cpool,
            tc.tile_pool(name="wpool", bufs=4) as wpool,
            tc.tile_pool(name="curp", bufs=2) as curpool,
            tc.tile_pool(name="psp", bufs=4, space="PSUM") as pspool,
        ):
            xT = cpool.tile([128, 8, COLS], bf, tag="xT")
            nc.sync.dma_start(out=xT[:], in_=xT_d[:])
            sA = cpool.tile([128, 16, COLS], bf, tag="sA")
            sB = cpool.tile([128, 16, COLS], bf, tag="sB")

            bt = {}
            for nm, d, mt in (
                ("b1", b1_d, 16), ("b2", b2_d, 16),
                ("b3", b3_d, 16), ("bo", bo_d, 8),
            ):
                bt[nm] = cpool.tile([128, mt], f32, tag=nm, name=nm)
                nc.sync.dma_start(out=bt[nm][:], in_=d[:])

            def gemm_chunk(tci, w_d, btile, KT, MT, rhs):
                """One 512-column chunk of a layer's matmul.

                Returns the SBUF cur tile [128, MT, CHW] in bf16 with the
                bias already added.
                """
                curt = curpool.tile([128, MT, CHW], bf, tag="cur")
                for mt in range(MT):
                    wt = wpool.tile([128, KT, 128], bf, tag="wt")
                    nc.sync.dma_start(out=wt[:], in_=w_d[:, mt])
                    ps = pspool.tile([128, CHW], f32, tag="ps")
                    for kt in range(KT):
                        nc.tensor.matmul(
                            ps[:],
                            wt[:, kt, :],
                            rhs(kt, tci),
                            start=(kt == 0),
                            stop=(kt == KT - 1),
                        )
                    nc.scalar.activation(
                        curt[:, mt, :], ps[:], AF.Identity,
                        bias=btile[:, mt:mt + 1], scale=1.0,
                    )
                return curt

            def hidden_layer(w_d, bname, KT, rhs, s_out, beta, th):
                MT = 16
                mem = cpool.tile([128, MT, BS], bf, tag="mem")
                nc.vector.memset(mem[:], 0.0)
                for tci in range(NTC):
                    curt = gemm_chunk(tci, w_d, bt[bname], KT, MT, rhs)
                    for ti in range(TPC):
                        t = tci * TPC + ti
                        cur_sl = curt[:, :, ti * BS:(ti + 1) * BS]
                        # mem = beta*mem + cur
                        nc.vector.scalar_tensor_tensor(
                            mem[:], mem[:], float(beta), cur_sl,
                            Al.mult, Al.add,
                        )
                        if t > 0:
                            sprev = s_out[:, :, (t - 1) * BS:t * BS]
                            # mem = (-th)*spk_prev + mem
                            nc.vector.scalar_tensor_tensor(
                                mem[:], sprev, float(-th), mem[:],
                                Al.mult, Al.add,
                            )
                        nc.vector.tensor_scalar(
                            s_out[:, :, t * BS:(t + 1) * BS], mem[:],
                            float(th), None, Al.is_gt,
                        )

            # ---- layer 1: x (1024) -> 2048, spikes into sA
            hidden_layer(
                w1_d, "b1", 8,
                lambda kt, tci: xT[:, kt, tci * CHW:(tci + 1) * CHW],
                sA, beta1, th1,
            )
            # ---- layer 2: sA -> 2048, spikes into sB
            hidden_layer(
                w2_d, "b2", 16,
                lambda kt, tci: sA[:, kt, tci * CHW:(tci + 1) * CHW],
                sB, beta2, th2,
            )
            if debug:
                dbg = cpool.tile([128, 3, 16], f32, tag="dbg")
                nc.vector.tensor_reduce(
                    dbg[:, 0, :], sA[:], mybir.AxisListType.X, Al.add
                )
                nc.vector.tensor_reduce(
                    dbg[:, 1, :], sB[:], mybir.AxisListType.X, Al.add
                )
            # ---- layer 3: sB -> 2048, spikes into sA (reused)
            hidden_layer(
                w3_d, "b3", 16,
                lambda kt, tci: sB[:, kt, tci * CHW:(tci + 1) * CHW],
                sA, beta3, th3,
            )
            if debug:
                nc.vector.tensor_reduce(
                    dbg[:, 2, :], sA[:], mybir.AxisListType.X, Al.add
                )
                nc.sync.dma_start(out=dbg_d[:], in_=dbg[:])

            # ---- output layer: sA -> 1024 (1000 padded), accumulate spikes
            MT = 8
            memo = cpool.tile([128, MT, BS], bf, tag="memo")
            acc = cpool.tile([128, MT, BS], f32, tag="acc")
            spk0 = cpool.tile([128, MT, BS], bf, tag="spk0")
            spk1 = cpool.tile([128, MT, BS], bf, tag="spk1")
            spk = (spk0, spk1)
            nc.vector.memset(memo[:], 0.0)
            nc.vector.memset(acc[:], 0.0)
            for tci in range(NTC):
                curt = gemm_chunk(
                    tci, wo_d, bt["bo"], 16, MT,
                    lambda kt, _tci: sA[:, kt, _tci * CHW:(_tci + 1) * CHW],
                )
                for ti in range(TPC):
                    t = tci * TPC + ti
                    cur_sl = curt[:, :, ti * BS:(ti + 1) * BS]
                    nc.vector.scalar_tensor_tensor(
                        memo[:], memo[:], float(beta_o), cur_sl,
                        Al.mult, Al.add,
                    )
                    if t > 0:
                        nc.vector.scalar_tensor_tensor(
                            memo[:], spk[(t - 1) % 2][:], float(-th_o), memo[:],
                            Al.mult, Al.add,
                        )
                    nc.vector.tensor_scalar(
                        spk[t % 2][:], memo[:], float(th_o), None, Al.is_gt
                    )
                    nc.vector.tensor_tensor(acc[:], acc[:], spk[t % 2][:], Al.add)

            nc.sync.dma_start(out=out_d[:], in_=acc[:])

    nc.compile()
    return nc


def _get_compiled(params, debug=False):
    key = (params, debug)
    if key not in _COMPILED:
        _COMPILED[key] = _build(params, debug=debug)
    return _COMPILED[key]


# --------------------------------------------------------------------------
# Host-side data prep
# --------------------------------------------------------------------------

def _block_weights(w, KT, MT):
    """[M, K] fp32 -> [128, MT, KT, 128] bf16 with out[p, mt, kt, f] =
    w[mt*128 + f, kt*128 + p]."""
    M, K = w.shape
    assert M == MT * 128 and K == KT * 128
    return np.ascontiguousarray(
        w.reshape(MT, 128, KT, 128).transpose(3, 0, 2, 1)
    ).astype(BF16)


def _prep_inputs(inputs):
    x = np.asarray(inputs["x_seq"], np.float32)
    w1 = np.asarray(inputs["w1"], np.float32)
    w2 = np.asarray(inputs["w2"], np.float32)
    w3 = np.asarray(inputs["w3"], np.float32)
    wo = np.asarray(inputs["wo"], np.float32)

    wo_p = np.zeros((1024, D_H), np.float32)
    wo_p[:D_OUT] = wo

    shared = {
        "w1T": _block_weights(w1, 8, 16),
        "w2T": _block_weights(w2, 16, 16),
        "w3T": _block_weights(w3, 16, 16),
        "woT": _block_weights(wo_p, 16, 8),
    }
    for nm, b, mt in (("b1v", inputs["b1"], 16), ("b2v", inputs["b2"], 16),
                      ("b3v", inputs["b3"], 16)):
        shared[nm] = np.ascontiguousarray(
            np.asarray(b, np.float32).reshape(mt, 128).T
        )
    bo_p = np.zeros(1024, np.float32)
    bo_p[:D_OUT] = np.asarray(inputs["bo"], np.float32)
    shared["bov"] = np.ascontiguousarray(bo_p.reshape(8, 128).T)

    # per-core x: [p, kt, col] with col = t*16 + b
    xs = []
    xr = x.reshape(T, NCORES, BS, 8, 128)      # [t, c, b, kt, p]
    for c in range(NCORES):
        xc = xr[:, c].transpose(3, 2, 0, 1).reshape(128, 8, COLS)
        xs.append(np.ascontiguousarray(xc).astype(BF16))
    return shared, xs


def _params_from_inputs(inputs):
    def f(v):
        return float(np.asarray(v, np.float32))
    return (
        float(np.clip(f(inputs["beta1"]), 0.0, 1.0)), f(inputs["th1"]),
        float(np.clip(f(inputs["beta2"]), 0.0, 1.0)), f(inputs["th2"]),
        float(np.clip(f(inputs["beta3"]), 0.0, 1.0)), f(inputs["th3"]),
        float(np.clip(f(inputs["beta_out"]), 0.0, 1.0)), f(inputs["th_out"]),
    )


def _assemble_output(results):
    out = np.zeros((B, D_OUT), np.float32)
    for c in range(NCORES):
        a = np.asarray(results[c]["acc_out"], np.float32)   # [128, 8, 16]
        out[c * BS:(c + 1) * BS] = (
            a.transpose(2, 1, 0).reshape(BS, 1024)[:, :D_OUT]
        )
    return out


# --------------------------------------------------------------------------
# Entry point
# --------------------------------------------------------------------------

def kernel(**inputs):
    from concourse.bass_utils import run_bass_kernel_spmd

    params = _params_from_inputs(inputs)
    debug = bool(int(os.environ.get("SNN_KERNEL_DEBUG", "0")))
    nc = _get_compiled(params, debug=debug)
    shared, xs = _prep_inputs(inputs)
    in_maps = [dict(shared, xT=xs[c]) for c in range(NCORES)]
    trace = bool(int(os.environ.get("SNN_KERNEL_TRACE", "0")))
    try:
        res = run_bass_kernel_spmd(
            nc, in_maps, list(range(NCORES)), trace=trace
        )
    except ModuleNotFoundError:
        res = run_bass_kernel_spmd(nc, in_maps, list(range(NCORES)))
    out = _assemble_output(res.results)
    kernel.last_results = res
    return out


# revision 45
# speedup vs baseline: 1.0515x; 1.0515x over previous
"""Trainium2 Bass kernel for the DenseSNN problem (4-layer LIF spiking MLP).

Strategy
--------
Data-parallel over batch: B=128 is split into 8 shards of 16, one per
NeuronCore, with weights replicated (no collectives at all).

Per core the time recurrence is restructured layer-at-a-time: layer l's
input spikes for ALL timesteps are known once layer l-1's LIF scan
finishes, so each layer becomes ONE batched matmul over all (t, b) pairs
(M = T*Bs = 1024 rows -> full PE utilization) followed by a sequential
64-step elementwise LIF scan on the Vector engine:

    mem   = beta*mem + cur[t] - th*spk[t-1]     (2x scalar_tensor_tensor)
    spk[t] = (mem > th)                          (tensor_scalar is_gt)

All matmul operands are bf16 (1 cycle/row on the PE vs 4 for fp32);
accumulation stays fp32 in PSUM. Spikes are exactly representable in
bf16 (0.0/1.0). The per-neuron bias is fused into the PSUM->SBUF
evacuation on the Scalar engine (activation Identity with a bias AP).

Layers are processed in two 512-column chunks (tc) so the next chunk's /
next layer's matmuls overlap the previous chunk's LIF scan.

Layout (per core)
-----------------
Columns are (t, b) pairs, t-major: col = t*16 + b, 1024 columns total.
Activations/spikes live as [128 partitions, kt, col]: row kt*128+p of the
conceptual [D, cols] matrix sits at partition p, block kt.
Weights are pre-transposed + blocked host-side to [p, mt, kt, f] so each
matmul lhsT tile is w[:, mt, kt, :] = [K=128, M=128] and a whole mt-chunk
is one contiguous DMA.
"""

import os
import sys

import numpy as np
import ml_dtypes

if "/opt/trn_rl_repo" not in sys.path:
    sys.path.insert(0, "/opt/trn_rl_repo")

T, B, D_IN, D_H, D_OUT = 64, 128, 1024, 2048, 1000
NCORES = 8
BS = B // NCORES           # 16 batch rows per core
COLS = T * BS              # 1024 (t, b) columns
NTC = 2                    # column chunks per layer
CHW = COLS // NTC          # 512 columns per chunk (one PSUM bank)
TPC = T // NTC             # timesteps per chunk

BF16 = ml_dtypes.bfloat16

_COMPILED = {}


# --------------------------------------------------------------------------
# Program construction
# --------------------------------------------------------------------------

def _build(params, debug=False):
    from concourse import bacc, tile, mybir

    beta1, th1, beta2, th2, beta3, th3, beta_o, th_o = params
    f32 = mybir.dt.float32
    bf = mybir.dt.bfloat16
    Al = mybir.AluOpType
    AF = mybir.ActivationFunctionType

    nc = bacc.Bacc(
        "TRN2", target_bir_lowering=False, debug=False, num_devices=NCORES
    )

    xT_d = nc.dram_tensor("xT", [128, T, 8, BS], bf, kind="ExternalInput")
    w1_d = nc.dram_tensor("w1T", [128, 16, 8, 128], bf, kind="ExternalInput")
    w2_d = nc.dram_tensor("w2T", [128, 16, 16, 128], bf, kind="ExternalInput")
    w3_d = nc.dram_tensor("w3T", [128, 16, 16, 128], bf, kind="ExternalInput")
    wo_d = nc.dram_tensor("woT", [128, 8, 16, 128], bf, kind="ExternalInput")
    b1_d = nc.dram_tensor("b1v", [128, 16], f32, kind="ExternalInput")
    b2_d = nc.dram_tensor("b2v", [128, 16], f32, kind="ExternalInput")
    b3_d = nc.dram_tensor("b3v", [128, 16], f32, kind="ExternalInput")
    bo_d = nc.dram_tensor("bov", [128, 8], f32, kind="ExternalInput")
    out_d = nc.dram_tensor("acc_out", [128, 8, BS], f32, kind="ExternalOutput")
    if debug:
        dbg_d = nc.dram_tensor("dbg_s", [128, 3, 16], f32, kind="ExternalOutput")

    with tile.TileContext(nc) as tc:
        with (
            tc.tile_pool(name="const", bufs=1) as cpool,
            tc.tile_pool(name="wpool", bufs=6) as wpool,
            tc.tile_pool(name="curp", bufs=3) as curpool,
            tc.tile_pool(name="psp", bufs=6, space="PSUM") as pspool,
        ):
            # Spikes and x live t-major [128, t, kt, b]: each scan step's
            # spike write is one contiguous 512B-per-partition block, so
            # Tile's byte-range dependency tracking stays precise and
            # next-layer matmuls can start as soon as the columns they read
            # exist (bounding boxes of strided writes would otherwise
            # serialize every matmul behind the whole scan).
            xT = cpool.tile([128, T, 8, BS], bf, tag="xT")
            bt = {}
            for nm, d, mt in (
                ("b1", b1_d, 16), ("b2", b2_d, 16),
                ("b3", b3_d, 16), ("bo", bo_d, 8),
            ):
                bt[nm] = cpool.tile([128, mt], f32, tag=nm, name=nm)
                nc.gpsimd.dma_start(out=bt[nm][:], in_=d[:])
            xq = (nc.gpsimd, nc.sync, nc.scalar)
            for q in range(4):
                xq[q % 3].dma_start(
                    out=xT[:, q * 16:(q + 1) * 16], in_=xT_d[:, q * 16:(q + 1) * 16]
                )
            sA = cpool.tile([128, T, 16, BS], bf, tag="sA")
            sB = cpool.tile([128, T, 16, BS], bf, tag="sB")

            def gemm_chunk(c0, cw, w_d, btile, KT, MT, rhs):
                """One column chunk [c0, c0+cw) of a layer's matmul.

                Returns the SBUF cur tile [128, cw//BS, MT*BS] (t-major so
                the per-timestep scan slices are flat contiguous) in bf16
                with the bias already added.

                Weights/bias arrive pre-scaled by -1/th on the host, so the
                tile holds c̃ = -cur/th and the LIF scan runs on the negated
                membrane m̃ = -mem/th (threshold crossing = m̃ < -1).
                """
                nt = cw // BS
                curt = curpool.tile([128, nt, MT * BS], bf, tag="cur")
                for mt in range(MT):
                    wt = wpool.tile([128, KT, 128], bf, tag="wt")
                    # alternate DMA queues: one queue sustains ~150 GB/s and
                    # the weight restream needs more than that to stay ahead
                    weng = nc.sync if mt % 2 == 0 else nc.scalar
                    weng.dma_start(out=wt[:], in_=w_d[:, mt])
                    ps = pspool.tile([128, cw], f32, tag="ps")
                    for kt in range(KT):
                        nc.tensor.matmul(
                            ps[:],
                            wt[:, kt, :],
                            rhs(kt, c0, cw),
                            start=(kt == 0),
                            stop=(kt == KT - 1),
                        )
                    nc.scalar.activation(
                        curt[:, :, mt * BS:(mt + 1) * BS], ps[:], AF.Identity,
                        bias=btile[:, mt:mt + 1], scale=1.0,
                    )
                return curt

            def lif_step(mem2, mtmp, t, cur_sl, beta, spike_out, spike_prev):
                """One LIF timestep on the negated membrane m̃ = -mem/th.

                    m̃(t)  = beta*m̃(t-1) + c̃(t) + spk(t-1)
                    spk(t) = (m̃(t) < -1)

                Three DVE ops, none in-place (in-place costs ~+90ns/op):
                A (STT) writes the scratch tile mtmp, B (TT, has a 2x bf16
                uop unlike STT) adds the previous spikes into the ping-pong
                state tile, TS emits the spikes.
                """
                mprev, mcur = mem2[(t + 1) % 2], mem2[t % 2]
                if t == 0:
                    nc.vector.scalar_tensor_tensor(
                        mcur[:], mprev[:], float(beta), cur_sl, Al.mult, Al.add,
                    )
                else:
                    nc.vector.scalar_tensor_tensor(
                        mtmp[:], mprev[:], float(beta), cur_sl, Al.mult, Al.add,
                    )
                    nc.vector.tensor_tensor(
                        mcur[:], mtmp[:], spike_prev, Al.add,
                    )
                nc.vector.tensor_scalar(
                    spike_out, mcur[:], -1.0, None, Al.is_lt,
                )

            def hidden_layer(li, w_d, bname, KT, rhs, s_out, beta,
                             chunks=((0, 512), (512, 512))):
                MT = 16
                mem2 = (
                    cpool.tile([128, MT * BS], bf, tag="mem0", name=f"mem0_{li}"),
                    cpool.tile([128, MT * BS], bf, tag="mem1", name=f"mem1_{li}"),
                )
                mtmp = cpool.tile(
                    [128, MT * BS], bf, tag="mtmp", name=f"mtmp_{li}"
                )
                nc.vector.memset(mem2[1][:], 0.0)
                for c0, cw in chunks:
                    curt = gemm_chunk(c0, cw, w_d, bt[bname], KT, MT, rhs)
                    for ti in range(cw // BS):
                        t = c0 // BS + ti
                        lif_step(
                            mem2, mtmp, t, curt[:, ti], beta,
                            s_out[:, t],
                            s_out[:, t - 1] if t else None,
                        )

            def rhs_of(s):
                return lambda kt, c0, cw: s[:, c0 // BS:(c0 + cw) // BS, kt, :]

            # ---- layer 1: x (1024) -> 2048, spikes into sA
            hidden_layer(1, w1_d, "b1", 8, rhs_of(xT), sA, beta1)
            # ---- layer 2: sA -> 2048, spikes into sB
            hidden_layer(2, w2_d, "b2", 16, rhs_of(sA), sB, beta2)
            if debug:
                dbg = cpool.tile([128, 3, 16], f32, tag="dbg")
                nc.vector.tensor_reduce(
                    dbg[:, 0, :], sA[:].rearrange("p t h b -> p h t b"),
                    mybir.AxisListType.XY, Al.add,
                )
                nc.vector.tensor_reduce(
                    dbg[:, 1, :], sB[:].rearrange("p t h b -> p h t b"),
                    mybir.AxisListType.XY, Al.add,
                )
            # ---- layer 3: sB -> 2048, spikes into sA (reused)
            hidden_layer(3, w3_d, "b3", 16, rhs_of(sB), sA, beta3)
            if debug:
                nc.vector.tensor_reduce(
                    dbg[:, 2, :], sA[:].rearrange("p t h b -> p h t b"),
                    mybir.AxisListType.XY, Al.add,
                )
                nc.sync.dma_start(out=dbg_d[:], in_=dbg[:])

            # ---- output layer: sA -> 1024 (1000 padded), accumulate spikes.
            # Uneven column chunks (512/384/128) so only the last 8 timesteps
            # of LIF scan serialize after the final matmul.
            MT = 8
            memo2 = (
                cpool.tile([128, MT * BS], bf, tag="memo0", name="memo0"),
                cpool.tile([128, MT * BS], bf, tag="memo1", name="memo1"),
            )
            spko = [
                cpool.tile([128, MT * BS], bf, tag=f"spko{i}", name=f"spko{i}")
                for i in range(8)
            ]
            acc = cpool.tile([128, MT * BS], f32, tag="acc")
            mtmpo = cpool.tile([128, MT * BS], bf, tag="mtmpo", name="mtmpo")
            nc.vector.memset(memo2[1][:], 0.0)
            nc.gpsimd.memset(acc[:], 0.0)
            for c0, cw in ((0, 448), (448, 448), (896, 128)):
                curt = gemm_chunk(c0, cw, wo_d, bt["bo"], 16, MT, rhs_of(sA))
                for ti in range(cw // BS):
                    t = c0 // BS + ti
                    lif_step(
                        memo2, mtmpo, t, curt[:, ti], beta_o,
                        spko[t % 8][:],
                        spko[(t - 1) % 8][:] if t else None,
                    )
                    # acc += spk on GpSimd; 8 spike slots give it 8 steps of
                    # slack so it never gates the DVE scan chain
                    nc.gpsimd.tensor_tensor(
                        acc[:], acc[:], spko[t % 8][:], Al.add
                    )

            nc.sync.dma_start(out=out_d[:], in_=acc[:])

    nc.compile()
    return nc


def _get_compiled(params, debug=False):
    key = (params, debug)
    if key not in _COMPILED:
        _COMPILED[key] = _build(params, debug=debug)
    return _COMPILED[key]


# --------------------------------------------------------------------------
# Host-side data prep
# --------------------------------------------------------------------------

def _block_weights(w, KT, MT):
    """[M, K] fp32 -> [128, MT, KT, 128] bf16 with out[p, mt, kt, f] =
    w[mt*128 + f, kt*128 + p]."""
    M, K = w.shape
    assert M == MT * 128 and K == KT * 128
    return np.ascontiguousarray(
        w.reshape(MT, 128, KT, 128).transpose(3, 0, 2, 1)
    ).astype(BF16)


def _prep_inputs(inputs):
    x = np.asarray(inputs["x_seq"], np.float32)

    # The kernel runs the LIF scan on the negated membrane m̃ = -mem/th, so
    # every layer's weights/bias are pre-scaled by -1/th (exact sign flip
    # when th == 1).
    ths = {k: float(np.asarray(inputs[k], np.float32))
           for k in ("th1", "th2", "th3", "th_out")}
    for k, v in ths.items():
        assert v > 0, f"negated-membrane transform requires {k} > 0, got {v}"

    w1 = np.asarray(inputs["w1"], np.float32) * (-1.0 / ths["th1"])
    w2 = np.asarray(inputs["w2"], np.float32) * (-1.0 / ths["th2"])
    w3 = np.asarray(inputs["w3"], np.float32) * (-1.0 / ths["th3"])
    wo = np.asarray(inputs["wo"], np.float32) * (-1.0 / ths["th_out"])

    wo_p = np.zeros((1024, D_H), np.float32)
    wo_p[:D_OUT] = wo

    shared = {
        "w1T": _block_weights(w1, 8, 16),
        "w2T": _block_weights(w2, 16, 16),
        "w3T": _block_weights(w3, 16, 16),
        "woT": _block_weights(wo_p, 16, 8),
    }
    for nm, b, thk, mt in (
        ("b1v", inputs["b1"], "th1", 16),
        ("b2v", inputs["b2"], "th2", 16),
        ("b3v", inputs["b3"], "th3", 16),
    ):
        shared[nm] = np.ascontiguousarray(
            (np.asarray(b, np.float32) * (-1.0 / ths[thk])).reshape(mt, 128).T
        )
    bo_p = np.zeros(1024, np.float32)
    bo_p[:D_OUT] = np.asarray(inputs["bo"], np.float32) * (-1.0 / ths["th_out"])
    shared["bov"] = np.ascontiguousarray(bo_p.reshape(8, 128).T)

    # per-core x, t-major: [p, t, kt, b]
    xs = []
    xr = x.reshape(T, NCORES, BS, 8, 128)      # [t, c, b, kt, p]
    for c in range(NCORES):
        xc = xr[:, c].transpose(3, 0, 2, 1)    # [p, t, kt, b]
        xs.append(np.ascontiguousarray(xc).astype(BF16))
    return shared, xs


def _params_from_inputs(inputs):
    def f(v):
        return float(np.asarray(v, np.float32))
    return (
        float(np.clip(f(inputs["beta1"]), 0.0, 1.0)), f(inputs["th1"]),
        float(np.clip(f(inputs["beta2"]), 0.0, 1.0)), f(inputs["th2"]),
        float(np.clip(f(inputs["beta3"]), 0.0, 1.0)), f(inputs["th3"]),
        float(np.clip(f(inputs["beta_out"]), 0.0, 1.0)), f(inputs["th_out"]),
    )


def _assemble_output(results):
    out = np.zeros((B, D_OUT), np.float32)
    for c in range(NCORES):
        a = np.asarray(results[c]["acc_out"], np.float32)   # [128, 8, 16]
        out[c * BS:(c + 1) * BS] = (
            a.transpose(2, 1, 0).reshape(BS, 1024)[:, :D_OUT]
        )
    return out


# --------------------------------------------------------------------------
# Entry point
# --------------------------------------------------------------------------

def kernel(**inputs):
    from concourse.bass_utils import run_bass_kernel_spmd

    params = _params_from_inputs(inputs)
    debug = bool(int(os.environ.get("SNN_KERNEL_DEBUG", "0")))
    nc = _get_compiled(params, debug=debug)
    shared, xs = _prep_inputs(inputs)
    in_maps = [dict(shared, xT=xs[c]) for c in range(NCORES)]
    trace = bool(int(os.environ.get("SNN_KERNEL_TRACE", "0")))
    try:
        res = run_bass_kernel_spmd(
            nc, in_maps, list(range(NCORES)), trace=trace
        )
    except ModuleNotFoundError:
        res = run_bass_kernel_spmd(nc, in_maps, list(range(NCORES)))
    out = _assemble_output(res.results)
    kernel.last_results = res
    return out
